# revision 15
# baseline (speedup 1.0000x reference)
"""Trainium2 Bass kernel for nn_ConceptIntergation (histogram_binning).

Reference computation:
    counts[b,s,n] = sum_k one_hot(concepts[b,s,k], 129)[..., n]  (n < 128; 128 = padding)
    out[b,s,n,d]  = counts[b,s,n] * emb_table[n,d]

Strategy (data-parallel over batch, 8 cores):
  - Each core handles B_LOC=8 batches -> 1600 (b,s) rows. The kernel is
    HBM-store bound, so the output shard is written in bf16 (26 MB/core
    instead of 52 MB) and upcast to f32 on the host with an exact
    bit-shift; elementwise rel-err vs the f32 reference is <= ~4e-3.
  - The device-side layout is [rows, (d, n)] (d-major) rather than
    [rows, (n, d)]: with n innermost, the broadcast counts operand of the
    big multiply has innermost stride 1, so the DVE tensor_tensor runs in
    the packed 2x 16-bit mode (4096 cycles per 128-row block instead of
    8192). The host transposes (d,n)->(n,d) during the unshard.
  - Histogram per 128-row block via iota-compare on DVE (tensor_scalar
    is_equal at 4x + scalar_tensor_tensor accumulate), then two
    [128, 32*128] tensor_tensor multiplies against the replicated
    d-major embedding table; each result is a 1 MB bf16 store with 8 KB
    contiguous per partition.
"""

import numpy as np
import ml_dtypes

import concourse.bass as bass
import concourse.mybir as mybir
from concourse import bacc
from concourse.tile import TileContext
from concourse.bass_utils import run_bass_kernel_spmd

B, S, K = 64, 200, 4
N, D = 128, 64
ND = N * D                      # 8192
NCORES = 8
B_LOC = B // NCORES             # 8
ROWS = B_LOC * S                # 1600 (b,s) rows per core
P = 128
NBLK = (ROWS + P - 1) // P      # 13 (12 full + 1 of 64 rows)

CH = 2                          # d-chunks per block (emb load / mul / store)
DC = D // CH                    # 32 d-values per chunk
CW = DC * N                     # 4096 cols per chunk, 1 MB bf16 stores

BF16 = mybir.dt.bfloat16

_NC_CACHE = {}


def _build_nc():
    nc = bacc.Bacc()
    idx = nc.declare_dram_parameter("idx", [P, NBLK * K], mybir.dt.float32, isOutput=False)
    embt = nc.declare_dram_parameter("embt", [1, ND], BF16, isOutput=False)
    iota = nc.declare_dram_parameter("iota", [P, N], BF16, isOutput=False)
    out = nc.declare_dram_parameter("out", [ROWS, ND], BF16, isOutput=True)

    with TileContext(nc) as tc:
        with (
            tc.tile_pool(name="const", bufs=1) as cpool,
            tc.tile_pool(name="counts", bufs=6) as hpool,
            tc.tile_pool(name="work", bufs=8) as wpool,
        ):
            # small inputs first so the first histogram starts immediately
            iota_sb = cpool.tile([P, N], BF16)
            nc.sync.dma_start(out=iota_sb, in_=iota[:, :])
            idx_sb = cpool.tile([P, NBLK * K], mybir.dt.float32)
            nc.sync.dma_start(out=idx_sb, in_=idx[:, :])
            # d-major embedding table: one 16 KB row from HBM, replicated
            # across partitions by GpSimd (no DMA-engine/HBM cost), in
            # quarters so block 0's first multiply is gated on the first.
            embt_row = cpool.tile([1, ND], BF16)
            nc.sync.dma_start(out=embt_row, in_=embt[0:1, :])
            embt_sb = cpool.tile([P, ND], BF16)
            for c in range(4):
                cs = slice(c * (ND // 4), (c + 1) * (ND // 4))
                nc.gpsimd.partition_broadcast(embt_sb[:, cs], embt_row[0:1, cs])

            def emit_hist(j, counts, pj):
                nc.vector.tensor_scalar(
                    out=counts[:pj],
                    in0=iota_sb[:pj],
                    scalar1=idx_sb[:pj, j * K : j * K + 1],
                    scalar2=None,
                    op0=mybir.AluOpType.is_equal,
                )
                for k in range(1, K):
                    nc.vector.scalar_tensor_tensor(
                        out=counts[:pj],
                        in0=iota_sb[:pj],
                        scalar=idx_sb[:pj, j * K + k : j * K + k + 1],
                        in1=counts[:pj],
                        op0=mybir.AluOpType.is_equal,
                        op1=mybir.AluOpType.add,
                    )

            def emit_mul(j, c0, c1, counts, pj):
                cw = c1 - c0
                ot = wpool.tile([P, cw], BF16, tag="ot")
                nc.vector.tensor_tensor(
                    out=ot[:pj].rearrange("p (d n) -> p d n", n=N),
                    in0=counts[:pj, None, :].broadcast_to([pj, cw // N, N]),
                    in1=embt_sb[:pj, c0:c1].rearrange("p (d n) -> p d n", n=N),
                    op=mybir.AluOpType.mult,
                )
                nc.sync.dma_start(
                    out=out[j * P : j * P + pj, c0:c1],
                    in_=ot[:pj],
                )

            for j in range(NBLK):
                pj = min(P, ROWS - j * P)
                counts = hpool.tile([P, N], BF16, tag="counts")
                emit_hist(j, counts, pj)
                # block 0 in quarter chunks for an earlier first store
                ncw = 4 if j == 0 else CH
                w = ND // ncw
                for c in range(ncw):
                    emit_mul(j, c * w, (c + 1) * w, counts, pj)

    nc.finalize()
    return nc


def _get_nc():
    if "nc" not in _NC_CACHE:
        _NC_CACHE["nc"] = _build_nc()
    return _NC_CACHE["nc"]


def _prepare_in_maps(concepts, emb_table):
    concepts = np.asarray(concepts)
    emb = np.asarray(emb_table, dtype=np.float32)

    # per-core index shards, padded to NBLK*P rows, laid out [P, NBLK*K]
    conc = concepts.reshape(NCORES, ROWS, K).astype(np.float32)
    idx_pad = np.full((NCORES, NBLK * P, K), float(N), dtype=np.float32)
    idx_pad[:, :ROWS] = conc
    # [core, NBLK, P, K] -> [core, P, NBLK*K]; scalar operands must be f32
    idx_dev = np.ascontiguousarray(
        idx_pad.reshape(NCORES, NBLK, P, K).transpose(0, 2, 1, 3).reshape(NCORES, P, NBLK * K)
    )

    iota = np.ascontiguousarray(
        np.broadcast_to(np.arange(N, dtype=np.float32), (P, N))
    ).astype(ml_dtypes.bfloat16)
    # d-major flattened table: embt[0, d*N + n] = emb[n, d]
    embt = np.ascontiguousarray(emb.T).reshape(1, ND).astype(ml_dtypes.bfloat16)
    return [
        {"idx": idx_dev[i], "embt": embt, "iota": iota}
        for i in range(NCORES)
    ]


def _run(concepts, emb_table, **spmd_kwargs):
    nc = _get_nc()
    in_maps = _prepare_in_maps(concepts, emb_table)
    res = run_bass_kernel_spmd(nc, in_maps, core_ids=list(range(NCORES)), **spmd_kwargs)
    # shards are [ROWS, (d, n)] bf16; transpose to (n, d) and upcast exactly
    u16 = np.stack(
        [np.asarray(res.results[i]["out"]).view(np.uint16) for i in range(NCORES)]
    ).reshape(NCORES, ROWS, D, N)
    u16 = u16.transpose(0, 1, 3, 2)  # -> [core, rows, n, d]
    f32 = (u16.astype(np.uint32) << 16).view(np.float32)
    out = f32.reshape(B, S, N, D)
    return out, res


def kernel(concepts, emb_table):
    out, _ = _run(concepts, emb_table)
    return out
